# revision 39
# baseline (speedup 1.0000x reference)
"""Trainium2 Bass kernel for nn_AdaptiveMoELLM (2-layer MoE transformer with
lightning-indexer top-K attention and top-2-of-8 MoE routing, vocab head).

Distribution over 8 NeuronCores:
  - tokens (B*S = 2048) sharded 256/core for attention/norms/routing
    (cores 0-3 = batch 0, cores 4-7 = batch 1; AllGather of normalized
    activations within each 4-core batch group feeds full-sequence K/V)
  - experts sharded 1/core (dense token processing, gate-weighted,
    ReduceScatter returns summed per-token rows to their owners; gate
    columns routed to expert owners via AllToAll)
  - vocab projection column-sharded 4000/core after a final AllGather

Distribution: tokens sharded 256/core; attention K/V via one 4-core
AllGather per layer; MoE experts REPLICATED (each core streams all 8
experts' fp16 weights from HBM and processes its own tokens locally --
no MoE collectives); vocab column-sharded after a final AllGather.

Precision strategy (rel-err budget 2e-2; the reference's top-K masks and
top-2 router picks are extremely gap-sensitive, so discrete decisions
must match fp32 exactly): attention + scores in fp32 PE matmuls; layer-0
MoE in 3-pass fp16 hi/lo splits (fp16 products accumulate exactly in
fp32 PSUM => ~2^-21 accuracy, 3 cycles/row); layer-1 MoE single-pass
fp16 (its output feeds no further discrete decisions, only the vocab
head); vocab head and output in bf16.
"""

import numpy as np
import ml_dtypes

import concourse.bass as bass
import concourse.bacc as bacc
import concourse.mybir as mybir
import concourse.tile as tile
from concourse.bass import _add_dep_helper
from concourse.bass_utils import run_bass_kernel_spmd

F32 = mybir.dt.float32
F32R = mybir.dt.float32r
BF16 = mybir.dt.bfloat16
FP16 = mybir.dt.float16
I32 = mybir.dt.int32
AF = mybir.ActivationFunctionType
ALU = mybir.AluOpType
AX = mybir.AxisListType

L, D, H, DH, HI, DI, F, E = 2, 512, 8, 64, 4, 64, 2048, 8
V, S, B, K, TOPK_E = 32000, 1024, 2, 256, 2
NC = 8
TPC = 256
T = B * S
VPC = V // NC
EPS = 1e-5
N_ITERS = 23

bf16 = ml_dtypes.bfloat16


def _build(signs, dbg=False):
    nc = bacc.Bacc(None, num_devices=NC, debug=False, target_bir_lowering=False)

    def param(name, shape, dt):
        return nc.declare_dram_parameter(name, list(shape), dt, isOutput=False)

    x0_p = param("x0", [TPC, D], F32)
    c01_p = param("c01", [TPC, S], F32)
    idxq_p = param("idxq", [L, D, HI * DI], F32)
    idxk_p = param("idxk", [L, D, HI * DI], F32)
    wqkv_p = param("wqkv", [L, 4, D, D], F32)
    rw_p = param("rw", [L, D, E], F32)
    w1_p = param("w1", [L, E, D, F], FP16)
    w2_p = param("w2", [L, E, F, D], FP16)
    w1l_p = param("w1l", [E, D, F], FP16)
    w2l_p = param("w2l", [E, F, D], FP16)
    outw_p = param("outw", [D, VPC], BF16)
    idb_p = param("idb", [128, 128], BF16)
    one_p = param("one", [128, 64], F32)
    idf_p = param("idf", [128, 128], F32)
    out_p = nc.declare_dram_parameter("out", [T, VPC], BF16, isOutput=True)

    with tile.TileContext(nc) as tc:
        with (
            tc.tile_pool(name="cst", bufs=1) as cst,
            tc.tile_pool(name="wrk", bufs=2) as wrk,
            tc.tile_pool(name="sml", bufs=4) as sml,
            tc.tile_pool(name="ps", bufs=4, space="PSUM") as ps,
            tc.tile_pool(name="pst", bufs=2, space="PSUM") as pst,
            tc.tile_pool(name="dr", bufs=1, space="DRAM") as dr,
        ):
            # ---------------- persistent loads ----------------
            ident = cst.tile([128, 128], BF16)
            nc.sync.dma_start(ident[:], idb_p[:])
            identf = cst.tile([128, 128], F32)
            nc.sync.dma_start(identf[:], idf_p[:])
            ones64 = cst.tile([1, 64], F32)
            nc.sync.dma_start(ones64[:], one_p[0:1, :])
            x_own = cst.tile([128, 2, D], F32)
            nc.sync.dma_start(
                x_own[:], x0_p.rearrange("(t p) d -> p t d", p=128))
            c01 = cst.tile([128, 2, S], F32)
            c01_dma = nc.sync.dma_start(
                c01[:], c01_p.rearrange("(t p) k -> p t k", p=128))
            vals = cst.tile([128, 2, S], F32)
            ind = cst.tile([128, 2, S], BF16)
            indT = cst.tile([128, 8, TPC], BF16)

            def mm_ps(shape, pool=None, tag="mm", bufs=None):
                pool = pool or ps
                return pool.tile(shape, F32, tag=tag, bufs=bufs,
                                 name=f"ps_{tag}_{nc.next_id()}")

            def dump(name, ap):
                if not dbg:
                    return
                t = nc.declare_dram_parameter(
                    "dbg_" + name, list(ap.shape), ap.dtype, isOutput=True)
                nc.sync.dma_start(t[:], ap)

            def transpose_128(dst, src, dtype=F32):
                pt = pst.tile([128, 128], dtype, tag="tr",
                              name=f"pt_{nc.next_id()}")
                nc.tensor.transpose(
                    pt[:], src, ident[:] if dtype == BF16 else identf[:])
                nc.vector.tensor_copy(out=dst, in_=pt[:])

            def normalize(src_qt, dst_qt):
                """LayerNorm without affine (folded into consumers). f32."""
                ssum = sml.tile([128, 1], F32, tag="ln_s",
                                name=f"lns_{nc.next_id()}")
                nc.vector.tensor_reduce(
                    out=ssum[:], in_=src_qt, axis=AX.X, op=ALU.add)
                negmean = sml.tile([128, 1], F32, tag="ln_m",
                                   name=f"lnm_{nc.next_id()}")
                nc.vector.tensor_scalar(
                    out=negmean[:], in0=ssum[:], scalar1=-1.0 / D,
                    scalar2=None, op0=ALU.mult)
                xc = wrk.tile([128, D], F32, tag="ln_xc", bufs=1,
                              name=f"lnxc_{nc.next_id()}")
                var = sml.tile([128, 1], F32, tag="ln_v",
                               name=f"lnv_{nc.next_id()}")
                nc.vector.scalar_tensor_tensor(
                    out=xc[:], in0=src_qt, scalar=negmean[:], in1=src_qt,
                    op0=ALU.add, op1=ALU.bypass)
                sq = wrk.tile([128, D], F32, tag="ln_sq", bufs=1,
                              name=f"lnsq_{nc.next_id()}")
                nc.vector.scalar_tensor_tensor(
                    out=sq[:], in0=xc[:], scalar=1.0, in1=xc[:],
                    op0=ALU.mult, op1=ALU.mult, accum_out=var[:])
                vmean = sml.tile([128, 1], F32, tag="ln_vm",
                                 name=f"lnvm_{nc.next_id()}")
                nc.vector.tensor_scalar(
                    out=vmean[:], in0=var[:], scalar1=1.0 / D, scalar2=EPS,
                    op0=ALU.mult, op1=ALU.add)
                std = sml.tile([128, 1], F32, tag="ln_sd",
                               name=f"lnsd_{nc.next_id()}")
                nc.scalar.activation(std[:], vmean[:], AF.Sqrt)
                rstd = sml.tile([128, 1], F32, tag="ln_r",
                                name=f"lnr_{nc.next_id()}")
                nc.vector.reciprocal(rstd[:], std[:])
                # one Newton step: r' = r*(1.5 - 0.5*v*r^2)  (Sqrt ULP hedge)
                r2 = sml.tile([128, 1], F32, tag="ln_r2",
                              name=f"lnr2_{nc.next_id()}")
                nc.vector.tensor_tensor(out=r2[:], in0=rstd[:], in1=rstd[:],
                                        op=ALU.mult)
                vr2 = sml.tile([128, 1], F32, tag="ln_vr",
                               name=f"lnvr_{nc.next_id()}")
                nc.vector.tensor_tensor(out=vr2[:], in0=vmean[:], in1=r2[:],
                                        op=ALU.mult)
                nc.vector.tensor_scalar(
                    out=vr2[:], in0=vr2[:], scalar1=-0.5, scalar2=1.5,
                    op0=ALU.mult, op1=ALU.add)
                nc.vector.tensor_tensor(out=rstd[:], in0=rstd[:], in1=vr2[:],
                                        op=ALU.mult)
                nc.vector.tensor_scalar(
                    out=dst_qt, in0=xc[:], scalar1=rstd[:], scalar2=None,
                    op0=ALU.mult)

            # =======================================================
            last_wdma = [None]
            for l in range(L):
                xh = wrk.tile([128, 2, D], F32, tag="xh", bufs=1,
                              name=f"xh_{l}")
                for qt in range(2):
                    normalize(x_own[:, qt, :], xh[:, qt, :])

                with (
                    tc.tile_pool(name=f"attn{l}", bufs=1) as ab,
                    tc.tile_pool(name=f"aops{l}", bufs=2, space="PSUM") as aops,
                ):
                    idxq_sb = ab.tile([128, 4, HI * DI], F32, tag="idxq",
                                      name=f"idxq_{l}")
                    idxq_dma = nc.sync.dma_start(
                        idxq_sb[:],
                        idxq_p[l].rearrange("(d p) n -> p d n", p=128))
                    idxk_sb = ab.tile([128, 4, HI * DI], F32, tag="idxk",
                                      name=f"idxk_{l}")
                    idxk_dma = nc.sync.dma_start(
                        idxk_sb[:],
                        idxk_p[l].rearrange("(d p) n -> p d n", p=128))
                    wqkv_sb = ab.tile([128, 4, 4, D], F32, tag="wqkv",
                                      name=f"wqkv_{l}")
                    wqkv_dma = nc.sync.dma_start(
                        wqkv_sb[:],
                        wqkv_p[l].rearrange("m (d p) n -> p m d n", p=128))
                    wo_sb = ab.tile([64, H, D], F32, tag="wo_sb",
                                    name=f"wo_{l}")
                    wo_dma = nc.sync.dma_start(
                        wo_sb[:],
                        wqkv_p[l, 3].rearrange("(h p) n -> p h n", p=64))

                    hT_own = ab.tile([128, 4, TPC], F32, tag="hT_own",
                                     name=f"hTo_{l}")
                    for qt in range(2):
                        for dt in range(4):
                            transpose_128(
                                hT_own[:, dt, qt * 128:(qt + 1) * 128],
                                xh[:, qt, dt * 128:(dt + 1) * 128])
                    dump(f"hTo{l}", hT_own[:])
                    # q-side projections only need local hT_own; issue them
                    # early so PE works while AG1 is in flight
                    qiT_l = []
                    for hp in range(HI // 2):
                        qiT = ab.tile([128, TPC], F32, tag="qiT", bufs=2,
                                      name=f"qiT_{nc.next_id()}")
                        pq = mm_ps([128, TPC])
                        for dt in range(4):
                            nc.tensor.matmul(
                                pq[:],
                                (idxq_sb[:, dt, hp * 128:(hp + 1) * 128]),
                                (hT_own[:, dt, :]), start=dt == 0,
                                stop=dt == 3)
                        nc.scalar.copy(qiT[:], pq[:])
                        qiT_l.append(qiT)
                    qhT_l = []
                    for hp in range(H // 2):
                        qhT = ab.tile([128, TPC], F32, tag="qhT", bufs=4,
                                      name=f"qhT_{nc.next_id()}")
                        pq = mm_ps([128, TPC])
                        for dt in range(4):
                            nc.tensor.matmul(
                                pq[:],
                                (wqkv_sb[:, 0, dt, hp * 128:(hp + 1) * 128]),
                                (hT_own[:, dt, :]), start=dt == 0,
                                stop=dt == 3)
                        nc.scalar.copy(qhT[:], pq[:])
                        qhT_l.append(qhT)
                    ag1_in = dr.tile([D, TPC], F32, tag="ag1i",
                                     name=f"ag1i_{l}")
                    ag1in_dma = nc.sync.dma_start(
                        ag1_in.rearrange("(d p) t -> p d t", p=128), hT_own[:])
                    # prioritize the AllGather input over bulky weight loads
                    deps = [idxq_dma, idxk_dma, wqkv_dma, wo_dma]
                    if l == 0:
                        deps.append(c01_dma)
                    for _d in deps:
                        _add_dep_helper(_d.ins, ag1in_dma.ins, sync=True,
                                        reason="defer weight DMA behind ag1")
                    ag1_out = dr.tile([4 * D, TPC], F32, tag="ag1o",
                                      name=f"ag1o_{l}")
                    nc.gpsimd.collective_compute(
                        "AllGather", ALU.bypass,
                        ins=[ag1_in[:]], outs=[ag1_out[:]],
                        replica_groups=[[0, 1, 2, 3], [4, 5, 6, 7]])
                    hT_b = ab.tile([128, 4, S], F32, tag="hT_b",
                                   name=f"hTb_{l}")
                    for r in range(4):
                        nc.sync.dma_start(
                            hT_b[:, :, r * TPC:(r + 1) * TPC],
                            ag1_out[r * D:(r + 1) * D].rearrange(
                                "(d p) t -> p d t", p=128))
                    dump(f"hTb{l}", hT_b[:])

                    # ---- lightning indexer scores -> vals ----
                    for qt in range(2):
                        for ch in range(2):
                            nc.vector.tensor_scalar(
                                out=vals[:, qt, ch * 512:(ch + 1) * 512],
                                in0=c01[:, qt, ch * 512:(ch + 1) * 512],
                                scalar1=1e9, scalar2=-1e9,
                                op0=ALU.mult, op1=ALU.add)
                    for hp in range(HI // 2):
                        qiT = qiT_l[hp]
                        kiT = ab.tile([128, S], F32, tag="kiT", bufs=2,
                                      name=f"kiT_{nc.next_id()}")
                        for ch in range(2):
                            pk = mm_ps([128, 512])
                            for dt in range(4):
                                nc.tensor.matmul(
                                    pk[:],
                                    (idxk_sb[:, dt, hp * 128:(hp + 1) * 128]),
                                    (hT_b[:, dt, ch * 512:(ch + 1) * 512]),
                                    start=dt == 0, stop=dt == 3)
                            nc.scalar.copy(
                                kiT[:, ch * 512:(ch + 1) * 512], pk[:])
                        for hh in range(2):
                            h = hp * 2 + hh
                            for qt in range(2):
                                for ch in range(2):
                                    pv = mm_ps([128, 512])
                                    nc.tensor.matmul(
                                        pv[:],
                                        (qiT[hh * 64:(hh + 1) * 64,
                                               qt * 128:(qt + 1) * 128]),
                                        (kiT[hh * 64:(hh + 1) * 64,
                                               ch * 512:(ch + 1) * 512]),
                                        start=True, stop=True)
                                    rl = ab.tile([128, 512], F32, tag="rl",
                                                 bufs=3,
                                                 name=f"rl_{nc.next_id()}")
                                    nc.scalar.activation(rl[:], pv[:], AF.Relu)
                                    dst = vals[:, qt, ch * 512:(ch + 1) * 512]
                                    nc.vector.scalar_tensor_tensor(
                                        out=dst, in0=rl[:],
                                        scalar=float(signs[l][h]), in1=dst,
                                        op0=ALU.mult, op1=ALU.add)

                    # ---- top-K threshold: per-row binary search ----
                    lo = sml.tile([128, 2], F32, tag="lo", name=f"lo_{l}")
                    hi = sml.tile([128, 2], F32, tag="hi", name=f"hi_{l}")
                    for qt in range(2):
                        nc.vector.tensor_reduce(
                            out=hi[:, qt:qt + 1], in_=vals[:, qt, :],
                            axis=AX.X, op=ALU.max)
                        msk = ab.tile([128, S], F32, tag="msk", bufs=1,
                                      name=f"msk_{nc.next_id()}")
                        nc.vector.tensor_tensor(
                            out=msk[:], in0=vals[:, qt, :], in1=c01[:, qt, :],
                            op=ALU.mult)
                        nc.vector.tensor_reduce(
                            out=lo[:, qt:qt + 1], in_=msk[:], axis=AX.X,
                            op=ALU.min)
                    counts = sml.tile([128, 2], F32, tag="cnt",
                                      name=f"cnt_{l}")
                    for it in range(N_ITERS):
                        mid = sml.tile([128, 2], F32, tag="mid",
                                       name=f"mid_{nc.next_id()}")
                        nc.vector.tensor_tensor(
                            out=mid[:], in0=lo[:], in1=hi[:], op=ALU.add)
                        nc.vector.tensor_scalar(
                            out=mid[:], in0=mid[:], scalar1=0.5, scalar2=None,
                            op0=ALU.mult)
                        for qt in range(2):
                            junk = ab.tile([128, S], BF16, tag="junk", bufs=1,
                                           name=f"jk_{nc.next_id()}")
                            nc.vector.tensor_scalar(
                                out=junk[:], in0=vals[:, qt, :],
                                scalar1=mid[:, qt:qt + 1], scalar2=0.0,
                                op0=ALU.is_ge, op1=ALU.add,
                                accum_out=counts[:, qt:qt + 1])
                        hit = sml.tile([128, 2], I32, tag="hit",
                                       name=f"hit_{nc.next_id()}")
                        nc.vector.tensor_scalar(
                            out=hit[:], in0=counts[:], scalar1=float(K),
                            scalar2=None, op0=ALU.is_ge)
                        nc.vector.copy_predicated(lo[:], hit[:], mid[:])
                        nhit = sml.tile([128, 2], I32, tag="nhit",
                                        name=f"nh_{nc.next_id()}")
                        nc.vector.tensor_scalar(
                            out=nhit[:], in0=counts[:], scalar1=float(K),
                            scalar2=None, op0=ALU.is_lt)
                        nc.vector.copy_predicated(hi[:], nhit[:], mid[:])

                    for qt in range(2):
                        nc.vector.tensor_scalar(
                            out=ind[:, qt, :], in0=vals[:, qt, :],
                            scalar1=lo[:, qt:qt + 1], scalar2=None,
                            op0=ALU.is_ge)
                    dump(f"vals{l}", vals[:])
                    dump(f"lo{l}", lo[:])
                    dump(f"ind{l}", ind[:])
                    for qt in range(2):
                        for kt in range(8):
                            transpose_128(
                                indT[:, kt, qt * 128:(qt + 1) * 128],
                                ind[:, qt, kt * 128:(kt + 1) * 128],
                                dtype=BF16)

                    # ---- attention (f32r matmuls) ----
                    v_sb = ab.tile([128, 8, H, DH + 1], F32, tag="v_sb",
                                   name=f"v_{l}")
                    nc.sync.dma_start(
                        v_sb[:, :, :, DH:DH + 1],
                        one_p.rearrange("p (a b o) -> p a b o", a=8, o=1))
                    for kt in range(8):
                        pvv = mm_ps([128, 512])
                        for dt in range(4):
                            nc.tensor.matmul(
                                pvv[:],
                                (hT_b[:, dt, kt * 128:(kt + 1) * 128]),
                                (wqkv_sb[:, 2, dt, :]), start=dt == 0,
                                stop=dt == 3)
                        nc.vector.tensor_copy(
                            out=v_sb[:, kt, :, 0:DH],
                            in_=pvv[:].rearrange("p (h d) -> p h d", h=H))

                    # swapped PV: paT[dh, q] = sum_k v[k, dh] * p[k, q]
                    # gives attention output pre-transposed for the wo proj;
                    # row DH is the softmax denominator.
                    aoT = ab.tile([DH, 2, H, 128], F32, tag="aoT",
                                  name=f"aoT_{l}")
                    for hp in range(H // 2):
                        qhT = qhT_l[hp]
                        khT = ab.tile([128, S], F32, tag="khT", bufs=2,
                                      name=f"khT_{nc.next_id()}")
                        for ch in range(2):
                            pk = mm_ps([128, 512])
                            for dt in range(4):
                                nc.tensor.matmul(
                                    pk[:],
                                    (wqkv_sb[:, 1, dt,
                                               hp * 128:(hp + 1) * 128]),
                                    (hT_b[:, dt, ch * 512:(ch + 1) * 512]),
                                    start=dt == 0, stop=dt == 3)
                            nc.scalar.copy(
                                khT[:, ch * 512:(ch + 1) * 512], pk[:])
                        for hh in range(2):
                            h = hp * 2 + hh
                            pa = mm_ps([DH + 1, 2, 128], pool=aops, tag="ao")
                            for kt in range(8):
                                pl = mm_ps([128, TPC])
                                nc.tensor.matmul(
                                    pl[:],
                                    (khT[hh * 64:(hh + 1) * 64,
                                           kt * 128:(kt + 1) * 128]),
                                    (qhT[hh * 64:(hh + 1) * 64, :]),
                                    start=True, stop=True)
                                pT = ab.tile([128, TPC], F32, tag="pT", bufs=3,
                                             name=f"pT_{nc.next_id()}")
                                nc.scalar.activation(pT[:], pl[:], AF.Exp)
                                nc.gpsimd.tensor_tensor(
                                    out=pT[:], in0=pT[:], in1=indT[:, kt, :],
                                    op=ALU.mult)
                                nc.tensor.matmul(
                                    pa[:].rearrange("m a b -> m (a b)"),
                                    (v_sb[:, kt, h, :]),
                                    (pT[:]), start=kt == 0, stop=kt == 7)
                            # normalize: divide the 64 value rows by the
                            # denominator row (DH), broadcast via K=1 matmul
                            rrow = ab.tile([1, TPC], F32, tag="rrow", bufs=2,
                                           name=f"rrow_{nc.next_id()}")
                            with nc.allow_low_precision(
                                    reason="f32r rounding of softmax denom"):
                                nc.vector.reciprocal(
                                    rrow[:].rearrange("m (a b) -> m a b",
                                                      a=2),
                                    pa[DH:DH + 1, :, :])
                            rb = pst.tile([DH, TPC], F32, tag="tr",
                                          name=f"rb_{nc.next_id()}")
                            nc.tensor.matmul(
                                rb[:], (ones64[:]), (rrow[:]),
                                start=True, stop=True)
                            rbs = ab.tile([DH, TPC], F32, tag="rbs", bufs=3,
                                          name=f"rbs_{nc.next_id()}")
                            nc.scalar.copy(rbs[:], rb[:])
                            nc.vector.tensor_tensor(
                                out=aoT[:, :, h, :],
                                in0=pa[0:DH, :, :],
                                in1=rbs[:].rearrange("m (a b) -> m a b", a=2),
                                op=ALU.mult)
                    dump(f"aoT{l}", aoT[:])

                    for qt in range(2):
                        po = mm_ps([128, D])
                        for h in range(H):
                            nc.tensor.matmul(
                                po[:],
                                (aoT[:, qt, h, :]),
                                (wo_sb[:, h, :]),
                                start=h == 0, stop=h == 7)
                        nc.vector.tensor_tensor(
                            out=x_own[:, qt, :], in0=x_own[:, qt, :],
                            in1=po[:], op=ALU.add)
                dump(f"xattn{l}", x_own[:])

                # ---- MoE ----
                mh = wrk.tile([128, 2, D], F32, tag="xh", bufs=1,
                              name=f"mh_{l}")
                for qt in range(2):
                    normalize(x_own[:, qt, :], mh[:, qt, :])

                with (
                    tc.tile_pool(name=f"moe{l}", bufs=1) as mb,
                    tc.tile_pool(name=f"moew{l}", bufs=2) as mw,
                    tc.tile_pool(name=f"moeps{l}", bufs=2,
                                 space="PSUM") as mps,
                ):
                    rw_sb = mb.tile([128, 4, E], F32, tag="rw",
                                    name=f"rw_{l}")
                    nc.sync.dma_start(
                        rw_sb[:], rw_p[l].rearrange("(d p) n -> p d n", p=128))
                    mT_own = mb.tile([128, 4, TPC], F32, tag="mT_own",
                                     name=f"mTo_{l}")
                    for qt in range(2):
                        for dt in range(4):
                            transpose_128(
                                mT_own[:, dt, qt * 128:(qt + 1) * 128],
                                mh[:, qt, dt * 128:(dt + 1) * 128])

                    gate = wrk.tile([128, 2, E], F32, tag="gate", bufs=1,
                                    name=f"gate_{l}")
                    for qt in range(2):
                        pr = mm_ps([128, E])
                        for dt in range(4):
                            nc.tensor.matmul(
                                pr[:],
                                (mT_own[:, dt, qt * 128:(qt + 1) * 128]),
                                (rw_sb[:, dt, :]), start=dt == 0,
                                stop=dt == 3)
                        rl_ = sml.tile([128, E], F32, tag="rlog",
                                       name=f"rlog_{nc.next_id()}")
                        nc.vector.tensor_copy(out=rl_[:], in_=pr[:])
                        m1 = sml.tile([128, 1], F32, tag="m1",
                                      name=f"m1_{nc.next_id()}")
                        nc.vector.tensor_reduce(out=m1[:], in_=rl_[:],
                                                axis=AX.X, op=ALU.max)
                        t1 = sml.tile([128, E], F32, tag="t1",
                                      name=f"t1_{nc.next_id()}")
                        nc.vector.tensor_scalar(
                            out=t1[:], in0=rl_[:], scalar1=m1[:],
                            scalar2=None, op0=ALU.is_equal)
                        lp = sml.tile([128, E], F32, tag="lp",
                                      name=f"lp_{nc.next_id()}")
                        nc.vector.scalar_tensor_tensor(
                            out=lp[:], in0=t1[:], scalar=-1e30, in1=rl_[:],
                            op0=ALU.mult, op1=ALU.add)
                        m2 = sml.tile([128, 1], F32, tag="m2",
                                      name=f"m2_{nc.next_id()}")
                        nc.vector.tensor_reduce(out=m2[:], in_=lp[:],
                                                axis=AX.X, op=ALU.max)
                        dd = sml.tile([128, 1], F32, tag="dd",
                                      name=f"dd_{nc.next_id()}")
                        nc.vector.tensor_tensor(out=dd[:], in0=m1[:],
                                                in1=m2[:], op=ALU.subtract)
                        g1 = sml.tile([128, 1], F32, tag="g1",
                                      name=f"g1_{nc.next_id()}")
                        nc.scalar.activation(g1[:], dd[:], AF.Sigmoid)
                        g2 = sml.tile([128, 1], F32, tag="g2",
                                      name=f"g2_{nc.next_id()}")
                        nc.vector.tensor_scalar(
                            out=g2[:], in0=g1[:], scalar1=-1.0, scalar2=1.0,
                            op0=ALU.mult, op1=ALU.add)
                        t2 = sml.tile([128, E], F32, tag="t2",
                                      name=f"t2_{nc.next_id()}")
                        nc.vector.tensor_scalar(
                            out=t2[:], in0=lp[:], scalar1=m2[:], scalar2=None,
                            op0=ALU.is_equal)
                        nc.vector.tensor_scalar(
                            out=gate[:, qt, :], in0=t1[:], scalar1=g1[:],
                            scalar2=None, op0=ALU.mult)
                        nc.vector.scalar_tensor_tensor(
                            out=gate[:, qt, :], in0=t2[:], scalar=g2[:],
                            in1=gate[:, qt, :], op0=ALU.mult, op1=ALU.add)
                    dump(f"gate{l}", gate[:])

                    # local experts: every core runs its own 256 tokens
                    # through all 8 experts (f32r weights streamed from HBM),
                    # gate-weighted accumulation -- no collectives
                    yacc = mb.tile([128, 2, D], F32, tag="yacc",
                                   name=f"yacc_{l}")
                    m1 = mb.tile([128, 4, TPC], FP16, tag="m1",
                                 name=f"m1_{l}")
                    nc.vector.tensor_copy(out=m1[:], in_=mT_own[:])
                    if l == 0:
                        m2 = mb.tile([128, 4, TPC], FP16, tag="m2",
                                     name=f"m2_{l}")
                        nc.gpsimd.tensor_tensor(
                            out=m2[:], in0=mT_own[:], in1=m1[:],
                            op=ALU.subtract)
                    for e in range(E):
                        w1h = mw.tile([128, 4, F], FP16, tag="w1h",
                                      name=f"w1h_{l}_{e}")
                        nc.sync.dma_start(
                            w1h[:],
                            w1_p[l, e].rearrange("(d p) f -> p d f", p=128))
                        w2h = mw.tile([128, 16, D], FP16, tag="w2h",
                                      name=f"w2h_{l}_{e}")
                        w2h_dma = nc.sync.dma_start(
                            w2h[:],
                            w2_p[l, e].rearrange("(f p) d -> p f d", p=128))
                        if l == 1 and e == E - 1:
                            last_wdma[0] = w2h_dma
                        if l == 0:
                            w1lo = mw.tile([128, 4, F], FP16, tag="w1l",
                                           name=f"w1l_{e}")
                            nc.sync.dma_start(
                                w1lo[:],
                                w1l_p[e].rearrange("(d p) f -> p d f", p=128))
                            w2lo = mw.tile([128, 16, D], FP16, tag="w2l",
                                           name=f"w2l_{e}")
                            nc.sync.dma_start(
                                w2lo[:],
                                w2l_p[e].rearrange("(f p) d -> p f d", p=128))
                        h1h = mb.tile([128, 16, TPC], FP16, tag="h1h",
                                      bufs=2,
                                      name=f"h1h_{nc.next_id()}")
                        if l == 0:
                            h1l = mb.tile([128, 16, TPC], FP16, tag="h1l",
                                          bufs=1, name=f"h1l_{nc.next_id()}")
                        for ft in range(16):
                            ph = mm_ps([128, TPC],
                                       pool=mps if ft % 3 == 2 else None)
                            if l == 0:
                                passes = ([(w1h, m1)] * 4 + [(w1h, m2)] * 4
                                          + [(w1lo, m1)] * 4)
                                for i, (ww, mm) in enumerate(passes):
                                    dt = i % 4
                                    nc.tensor.matmul(
                                        ph[:],
                                        ww[:, dt, ft * 128:(ft + 1) * 128],
                                        mm[:, dt, :],
                                        start=i == 0, stop=i == 11)
                            else:
                                for dt in range(4):
                                    nc.tensor.matmul(
                                        ph[:],
                                        w1h[:, dt, ft * 128:(ft + 1) * 128],
                                        m1[:, dt, :],
                                        start=dt == 0, stop=dt == 3)
                            if l == 0:
                                h1f = wrk.tile([128, TPC], F32, tag="h1f",
                                               name=f"h1f_{nc.next_id()}")
                                nc.scalar.activation(h1f[:], ph[:],
                                                     AF.Gelu_apprx_tanh)
                                nc.vector.tensor_copy(out=h1h[:, ft, :],
                                                      in_=h1f[:])
                                nc.gpsimd.tensor_tensor(
                                    out=h1l[:, ft, :], in0=h1f[:],
                                    in1=h1h[:, ft, :], op=ALU.subtract)
                            else:
                                nc.scalar.activation(h1h[:, ft, :], ph[:],
                                                     AF.Gelu_apprx_tanh)
                        for qt in range(2):
                            ph2 = mm_ps([128, D])
                            if l == 0:
                                nmm = 0
                                for ft in range(16):
                                    for hh, ww in ((h1h, w2h), (h1l, w2h),
                                                   (h1h, w2lo)):
                                        nc.tensor.matmul(
                                            ph2[:],
                                            hh[:, ft,
                                               qt * 128:(qt + 1) * 128],
                                            ww[:, ft, :], start=nmm == 0,
                                            stop=nmm == 47)
                                        nmm += 1
                            else:
                                for ft in range(16):
                                    nc.tensor.matmul(
                                        ph2[:],
                                        h1h[:, ft, qt * 128:(qt + 1) * 128],
                                        w2h[:, ft, :], start=ft == 0,
                                        stop=ft == 15)
                            if e == 0:
                                nc.vector.tensor_scalar(
                                    out=yacc[:, qt, :], in0=ph2[:],
                                    scalar1=gate[:, qt, e:e + 1],
                                    scalar2=None, op0=ALU.mult)
                            else:
                                nc.vector.scalar_tensor_tensor(
                                    out=yacc[:, qt, :], in0=ph2[:],
                                    scalar=gate[:, qt, e:e + 1],
                                    in1=yacc[:, qt, :],
                                    op0=ALU.mult, op1=ALU.add)
                    dump(f"yacc{l}", yacc[:])
                    for qt in range(2):
                        nc.vector.tensor_tensor(
                            out=x_own[:, qt, :], in0=x_own[:, qt, :],
                            in1=yacc[:, qt, :], op=ALU.add)
                dump(f"xmoe{l}", x_own[:])

            # =======================================================
            # final LN + vocab projection (column-sharded, bf16)
            # =======================================================
            with tc.tile_pool(name="voc", bufs=1) as vb:
                xf = wrk.tile([128, 2, D], F32, tag="xh", bufs=1, name="xf")
                for qt in range(2):
                    normalize(x_own[:, qt, :], xf[:, qt, :])
                xfb = vb.tile([128, 2, D], BF16, tag="xfb", name="xfb")
                nc.scalar.copy(xfb[:], xf[:])
                xfT_own = vb.tile([128, 4, TPC], BF16, tag="xfT", name="xfT")
                for qt in range(2):
                    for dt in range(4):
                        transpose_128(xfT_own[:, dt, qt * 128:(qt + 1) * 128],
                                      xfb[:, qt, dt * 128:(dt + 1) * 128],
                                      dtype=BF16)
                # AllGather in two token halves: vocab matmuls for the
                # first half overlap the second transfer
                xfT_full = vb.tile([128, 4, NC, TPC], BF16, tag="xfT_full",
                                   name="xfTf")
                for half in range(2):
                    agi = dr.tile([D, 128], BF16, tag=f"ag3i{half}",
                                  name=f"ag3i{half}")
                    nc.sync.dma_start(
                        agi.rearrange("(d p) t -> p d t", p=128),
                        xfT_own[:, :, half * 128:(half + 1) * 128])
                    ago = dr.tile([NC * D, 128], BF16, addr_space="Shared",
                                  tag=f"ag3o{half}", name=f"ag3o{half}")
                    nc.gpsimd.collective_compute(
                        "AllGather", ALU.bypass, ins=[agi[:]],
                        outs=[ago[:]], replica_groups=[list(range(NC))])
                    for r in range(NC):
                        nc.sync.dma_start(
                            xfT_full[:, :, r,
                                     half * 128:(half + 1) * 128],
                            ago[r * D:(r + 1) * D].rearrange(
                                "(d p) t -> p d t", p=128))

                NVC = 8
                CW = VPC // NVC  # 500
                for vc in range(NVC):
                    owc = vb.tile([128, 4, CW], BF16, tag="outw", bufs=2,
                                  name=f"owc_{vc}")
                    owc_dma = nc.sync.dma_start(
                        owc[:],
                        outw_p[:, vc * CW:(vc + 1) * CW].rearrange(
                            "(d p) v -> p d v", p=128))
                    if vc < 2 and last_wdma[0] is not None:
                        _add_dep_helper(owc_dma.ins, last_wdma[0].ins,
                                        sync=True,
                                        reason="vocab weights after experts")
                    for qt in (list(range(0, 16, 2))
                               + list(range(1, 16, 2))):
                        r, hf = qt // 2, qt % 2
                        pv = mm_ps([128, CW])
                        for dt in range(4):
                            nc.tensor.matmul(
                                pv[:],
                                xfT_full[:, dt, r, hf * 128:(hf + 1) * 128],
                                owc[:, dt, :], start=dt == 0, stop=dt == 3)
                        oc = vb.tile([128, CW], BF16, tag="oc", bufs=6,
                                     name=f"oc_{nc.next_id()}")
                        if qt % 2 == 0:
                            nc.vector.tensor_copy(out=oc[:], in_=pv[:])
                        else:
                            nc.scalar.copy(oc[:], pv[:])
                        nc.sync.dma_start(
                            out_p[qt * 128:(qt + 1) * 128,
                                  vc * CW:(vc + 1) * CW], oc[:])

    nc.compile()
    return nc


# -------------------------------------------------------------- host side --
_CACHE = {}
_LAST_IN_MAPS = None


def _np(x, dt=np.float32):
    return np.ascontiguousarray(np.asarray(x), dtype=dt)


def kernel(**inputs):
    ids = _np(inputs["input_ids"], np.int64).reshape(B, S)
    tok_emb = _np(inputs["tok_emb"])
    pos_emb = _np(inputs["pos_emb"])
    ln1_g, ln1_b = _np(inputs["ln1_g"]), _np(inputs["ln1_b"])
    ln2_g, ln2_b = _np(inputs["ln2_g"]), _np(inputs["ln2_b"])
    lnf_g, lnf_b = _np(inputs["lnf_g"]), _np(inputs["lnf_b"])
    idx_qw, idx_qb = _np(inputs["idx_qw"]), _np(inputs["idx_qb"])
    idx_kw, idx_kb = _np(inputs["idx_kw"]), _np(inputs["idx_kb"])
    idx_hw = _np(inputs["idx_hw"])
    wq, bq = _np(inputs["wq"]), _np(inputs["bq"])
    wk, bk = _np(inputs["wk"]), _np(inputs["bk"])
    wv, bv = _np(inputs["wv"]), _np(inputs["bv"])
    wo, bo = _np(inputs["wo"]), _np(inputs["bo"])
    router_w, router_b = _np(inputs["router_w"]), _np(inputs["router_b"])
    e_w1, e_b1 = _np(inputs["e_w1"]), _np(inputs["e_b1"])
    e_w2, e_b2 = _np(inputs["e_w2"]), _np(inputs["e_b2"])
    out_w, out_b = _np(inputs["out_w"]), _np(inputs["out_b"])

    for nm, b in [("ln1_b", ln1_b), ("ln2_b", ln2_b), ("lnf_b", lnf_b),
                  ("idx_qb", idx_qb), ("idx_kb", idx_kb), ("bq", bq),
                  ("bk", bk), ("bv", bv), ("bo", bo), ("router_b", router_b),
                  ("e_b1", e_b1), ("e_b2", e_b2), ("out_b", out_b)]:
        assert np.abs(b).max() == 0.0, f"nonzero bias {nm} unsupported"

    x0 = tok_emb[ids.reshape(-1)] + np.tile(pos_emb[:S], (B, 1))  # [T, D]

    scale = 1.0 / np.sqrt(DH)
    idxq_f = idx_qw * ln1_g[:, :, None]
    signs = np.sign(idx_hw)
    signs[signs == 0] = 1.0
    for l in range(L):
        for h in range(HI):
            idxq_f[l][:, h * DI:(h + 1) * DI] *= abs(idx_hw[l, h])
    idxk_f = idx_kw * ln1_g[:, :, None]
    wq_f = wq * ln1_g[:, :, None] * scale
    wk_f = wk * ln1_g[:, :, None]
    wv_f = wv * ln1_g[:, :, None]
    wqkv = np.stack([wq_f, wk_f, wv_f, wo], axis=1)  # [L, 4, D, D]
    rw_f = router_w * ln2_g[:, :, None]
    w1_f = e_w1 * ln2_g[:, None, :, None]            # [L, E, D, F]
    outw_f = out_w * lnf_g[:, None]

    if "nc" not in _CACHE:
        _CACHE["nc"] = _build(signs)
    nc = _CACHE["nc"]

    ident_b = np.eye(128, dtype=bf16)
    ident_f = np.eye(128, dtype=np.float32)
    w1_hi = w1_f.astype(np.float16)
    w2_hi = e_w2.astype(np.float16)
    w1_lo = (w1_f[0] - w1_hi[0].astype(np.float64)).astype(np.float16)
    w2_lo = (e_w2[0] - w2_hi[0].astype(np.float64)).astype(np.float16)
    in_maps = []
    for c in range(NC):
        rows = slice(c * TPC, (c + 1) * TPC)
        p = np.arange(S)[(c % 4) * TPC:(c % 4 + 1) * TPC]
        c01 = (np.arange(S)[None, :] <= p[:, None]).astype(np.float32)
        in_maps.append({
            "x0": x0[rows].astype(np.float32),
            "c01": c01,
            "idxq": idxq_f.astype(np.float32),
            "idxk": idxk_f.astype(np.float32),
            "wqkv": wqkv.astype(np.float32),
            "rw": rw_f.astype(np.float32),
            "w1": w1_hi,
            "w2": w2_hi,
            "w1l": w1_lo,
            "w2l": w2_lo,
            "outw": outw_f[:, c * VPC:(c + 1) * VPC].astype(bf16),
            "idb": ident_b,
            "idf": ident_f,
            "one": np.ones((128, 64), np.float32),
        })

    global _LAST_IN_MAPS
    _LAST_IN_MAPS = in_maps
    res = run_bass_kernel_spmd(nc, in_maps, core_ids=list(range(NC)))
    outs = [np.asarray(res.results[c]["out"]).astype(np.float32)
            for c in range(NC)]
    full = np.concatenate(outs, axis=1).reshape(B, S, V)
    return np.ascontiguousarray(full, dtype=np.float32)


if __name__ == "__main__":
    import reference
    inp = {k: np.asarray(v) for k, v in reference.setup_inputs().items()}
    got = kernel(**inp)
    print("kernel output", got.shape, got.dtype)


# revision 40
# speedup vs baseline: 1.0075x; 1.0075x over previous
"""Trainium2 Bass kernel for nn_AdaptiveMoELLM (2-layer MoE transformer with
lightning-indexer top-K attention and top-2-of-8 MoE routing, vocab head).

Distribution over 8 NeuronCores:
  - tokens (B*S = 2048) sharded 256/core for attention/norms/routing
    (cores 0-3 = batch 0, cores 4-7 = batch 1; AllGather of normalized
    activations within each 4-core batch group feeds full-sequence K/V)
  - experts sharded 1/core (dense token processing, gate-weighted,
    ReduceScatter returns summed per-token rows to their owners; gate
    columns routed to expert owners via AllToAll)
  - vocab projection column-sharded 4000/core after a final AllGather

Distribution: tokens sharded 256/core; attention K/V via one 4-core
AllGather per layer; MoE experts REPLICATED (each core streams all 8
experts' fp16 weights from HBM and processes its own tokens locally --
no MoE collectives); vocab column-sharded after a final AllGather.

Precision strategy (rel-err budget 2e-2; the reference's top-K masks and
top-2 router picks are extremely gap-sensitive, so discrete decisions
must match fp32 exactly): attention + scores in fp32 PE matmuls; layer-0
MoE in 3-pass fp16 hi/lo splits (fp16 products accumulate exactly in
fp32 PSUM => ~2^-21 accuracy, 3 cycles/row); layer-1 MoE single-pass
fp16 (its output feeds no further discrete decisions, only the vocab
head); vocab head and output in bf16.
"""

import numpy as np
import ml_dtypes

import concourse.bass as bass
import concourse.bacc as bacc
import concourse.mybir as mybir
import concourse.tile as tile
from concourse.bass import _add_dep_helper
from concourse.bass_utils import run_bass_kernel_spmd

F32 = mybir.dt.float32
F32R = mybir.dt.float32r
BF16 = mybir.dt.bfloat16
FP16 = mybir.dt.float16
I32 = mybir.dt.int32
AF = mybir.ActivationFunctionType
ALU = mybir.AluOpType
AX = mybir.AxisListType

L, D, H, DH, HI, DI, F, E = 2, 512, 8, 64, 4, 64, 2048, 8
V, S, B, K, TOPK_E = 32000, 1024, 2, 256, 2
NC = 8
TPC = 256
T = B * S
VPC = V // NC
EPS = 1e-5
N_ITERS = 23

bf16 = ml_dtypes.bfloat16


def _build(signs, dbg=False):
    nc = bacc.Bacc(None, num_devices=NC, debug=False, target_bir_lowering=False)

    def param(name, shape, dt):
        return nc.declare_dram_parameter(name, list(shape), dt, isOutput=False)

    x0_p = param("x0", [TPC, D], F32)
    c01_p = param("c01", [TPC, S], F32)
    idxq_p = param("idxq", [L, D, HI * DI], F32)
    idxk_p = param("idxk", [L, D, HI * DI], F32)
    idxkh_p = param("idxkh", [L, D, HI * DI], BF16)
    idxkl_p = param("idxkl", [L, D, HI * DI], BF16)
    wkvh_p = param("wkvh", [L, 2, D, D], BF16)
    wkvl_p = param("wkvl", [L, 2, D, D], BF16)
    wqkv_p = param("wqkv", [L, 4, D, D], F32)
    rw_p = param("rw", [L, D, E], F32)
    w1_p = param("w1", [L, E, D, F], FP16)
    w2_p = param("w2", [L, E, F, D], FP16)
    w1l_p = param("w1l", [E, D, F], FP16)
    w2l_p = param("w2l", [E, F, D], FP16)
    outw_p = param("outw", [D, VPC], BF16)
    idb_p = param("idb", [128, 128], BF16)
    one_p = param("one", [128, 64], F32)
    idf_p = param("idf", [128, 128], F32)
    out_p = nc.declare_dram_parameter("out", [T, VPC], BF16, isOutput=True)

    with tile.TileContext(nc) as tc:
        with (
            tc.tile_pool(name="cst", bufs=1) as cst,
            tc.tile_pool(name="wrk", bufs=2) as wrk,
            tc.tile_pool(name="sml", bufs=4) as sml,
            tc.tile_pool(name="ps", bufs=4, space="PSUM") as ps,
            tc.tile_pool(name="pst", bufs=2, space="PSUM") as pst,
            tc.tile_pool(name="dr", bufs=1, space="DRAM") as dr,
        ):
            # ---------------- persistent loads ----------------
            ident = cst.tile([128, 128], BF16)
            nc.sync.dma_start(ident[:], idb_p[:])
            identf = cst.tile([128, 128], F32)
            nc.sync.dma_start(identf[:], idf_p[:])
            ones64 = cst.tile([1, 64], F32)
            nc.sync.dma_start(ones64[:], one_p[0:1, :])
            x_own = cst.tile([128, 2, D], F32)
            nc.sync.dma_start(
                x_own[:], x0_p.rearrange("(t p) d -> p t d", p=128))
            c01 = cst.tile([128, 2, S], F32)
            c01_dma = nc.sync.dma_start(
                c01[:], c01_p.rearrange("(t p) k -> p t k", p=128))
            vals = cst.tile([128, 2, S], F32)
            ind = cst.tile([128, 2, S], BF16)
            indT = cst.tile([128, 8, TPC], BF16)

            def mm_ps(shape, pool=None, tag="mm", bufs=None):
                pool = pool or ps
                return pool.tile(shape, F32, tag=tag, bufs=bufs,
                                 name=f"ps_{tag}_{nc.next_id()}")

            def dump(name, ap):
                if not dbg:
                    return
                t = nc.declare_dram_parameter(
                    "dbg_" + name, list(ap.shape), ap.dtype, isOutput=True)
                nc.sync.dma_start(t[:], ap)

            def transpose_128(dst, src, dtype=F32):
                pt = pst.tile([128, 128], dtype, tag="tr",
                              name=f"pt_{nc.next_id()}")
                nc.tensor.transpose(
                    pt[:], src, ident[:] if dtype == BF16 else identf[:])
                nc.vector.tensor_copy(out=dst, in_=pt[:])

            def normalize(src_qt, dst_qt):
                """LayerNorm without affine (folded into consumers). f32."""
                ssum = sml.tile([128, 1], F32, tag="ln_s",
                                name=f"lns_{nc.next_id()}")
                nc.vector.tensor_reduce(
                    out=ssum[:], in_=src_qt, axis=AX.X, op=ALU.add)
                negmean = sml.tile([128, 1], F32, tag="ln_m",
                                   name=f"lnm_{nc.next_id()}")
                nc.vector.tensor_scalar(
                    out=negmean[:], in0=ssum[:], scalar1=-1.0 / D,
                    scalar2=None, op0=ALU.mult)
                xc = wrk.tile([128, D], F32, tag="ln_xc", bufs=1,
                              name=f"lnxc_{nc.next_id()}")
                var = sml.tile([128, 1], F32, tag="ln_v",
                               name=f"lnv_{nc.next_id()}")
                nc.vector.scalar_tensor_tensor(
                    out=xc[:], in0=src_qt, scalar=negmean[:], in1=src_qt,
                    op0=ALU.add, op1=ALU.bypass)
                sq = wrk.tile([128, D], F32, tag="ln_sq", bufs=1,
                              name=f"lnsq_{nc.next_id()}")
                nc.vector.scalar_tensor_tensor(
                    out=sq[:], in0=xc[:], scalar=1.0, in1=xc[:],
                    op0=ALU.mult, op1=ALU.mult, accum_out=var[:])
                vmean = sml.tile([128, 1], F32, tag="ln_vm",
                                 name=f"lnvm_{nc.next_id()}")
                nc.vector.tensor_scalar(
                    out=vmean[:], in0=var[:], scalar1=1.0 / D, scalar2=EPS,
                    op0=ALU.mult, op1=ALU.add)
                std = sml.tile([128, 1], F32, tag="ln_sd",
                               name=f"lnsd_{nc.next_id()}")
                nc.scalar.activation(std[:], vmean[:], AF.Sqrt)
                rstd = sml.tile([128, 1], F32, tag="ln_r",
                                name=f"lnr_{nc.next_id()}")
                nc.vector.reciprocal(rstd[:], std[:])
                # one Newton step: r' = r*(1.5 - 0.5*v*r^2)  (Sqrt ULP hedge)
                r2 = sml.tile([128, 1], F32, tag="ln_r2",
                              name=f"lnr2_{nc.next_id()}")
                nc.vector.tensor_tensor(out=r2[:], in0=rstd[:], in1=rstd[:],
                                        op=ALU.mult)
                vr2 = sml.tile([128, 1], F32, tag="ln_vr",
                               name=f"lnvr_{nc.next_id()}")
                nc.vector.tensor_tensor(out=vr2[:], in0=vmean[:], in1=r2[:],
                                        op=ALU.mult)
                nc.vector.tensor_scalar(
                    out=vr2[:], in0=vr2[:], scalar1=-0.5, scalar2=1.5,
                    op0=ALU.mult, op1=ALU.add)
                nc.vector.tensor_tensor(out=rstd[:], in0=rstd[:], in1=vr2[:],
                                        op=ALU.mult)
                nc.vector.tensor_scalar(
                    out=dst_qt, in0=xc[:], scalar1=rstd[:], scalar2=None,
                    op0=ALU.mult)

            # =======================================================
            last_wdma = [None]
            for l in range(L):
                xh = wrk.tile([128, 2, D], F32, tag="xh", bufs=1,
                              name=f"xh_{l}")
                for qt in range(2):
                    normalize(x_own[:, qt, :], xh[:, qt, :])

                with (
                    tc.tile_pool(name=f"attn{l}", bufs=1) as ab,
                    tc.tile_pool(name=f"aops{l}", bufs=2, space="PSUM") as aops,
                ):
                    idxq_sb = ab.tile([128, 4, HI * DI], F32, tag="idxq",
                                      name=f"idxq_{l}")
                    idxq_dma = nc.sync.dma_start(
                        idxq_sb[:],
                        idxq_p[l].rearrange("(d p) n -> p d n", p=128))
                    wqkv_sb = ab.tile([128, 4, D], F32, tag="wqkv",
                                      name=f"wqkv_{l}")
                    wqkv_dma = nc.sync.dma_start(
                        wqkv_sb[:],
                        wqkv_p[l, 0].rearrange("(d p) n -> p d n", p=128))
                    wo_sb = ab.tile([64, H, D], F32, tag="wo_sb",
                                    name=f"wo_{l}")
                    wo_dma = nc.sync.dma_start(
                        wo_sb[:],
                        wqkv_p[l, 3].rearrange("(h p) n -> p h n", p=64))
                    ikh = ab.tile([128, 4, HI * DI], BF16, tag="ikh",
                                  name=f"ikh_{l}")
                    ikh_dma = nc.sync.dma_start(
                        ikh[:], idxkh_p[l].rearrange("(d p) n -> p d n",
                                                     p=128))
                    ikl = ab.tile([128, 4, HI * DI], BF16, tag="ikl",
                                  name=f"ikl_{l}")
                    ikl_dma = nc.sync.dma_start(
                        ikl[:], idxkl_p[l].rearrange("(d p) n -> p d n",
                                                     p=128))
                    kvh = ab.tile([128, 2, 4, D], BF16, tag="kvh",
                                  name=f"kvh_{l}")
                    kvh_dma = nc.sync.dma_start(
                        kvh[:], wkvh_p[l].rearrange("m (d p) n -> p m d n",
                                                    p=128))
                    kvl = ab.tile([128, 2, 4, D], BF16, tag="kvl",
                                  name=f"kvl_{l}")
                    kvl_dma = nc.sync.dma_start(
                        kvl[:], wkvl_p[l].rearrange("m (d p) n -> p m d n",
                                                    p=128))

                    hT_own = ab.tile([128, 4, TPC], F32, tag="hT_own",
                                     name=f"hTo_{l}")
                    for qt in range(2):
                        for dt in range(4):
                            transpose_128(
                                hT_own[:, dt, qt * 128:(qt + 1) * 128],
                                xh[:, qt, dt * 128:(dt + 1) * 128])
                    dump(f"hTo{l}", hT_own[:])
                    # q-side projections only need local hT_own; issue them
                    # early so PE works while AG1 is in flight
                    qiT_l = []
                    for hp in range(HI // 2):
                        qiT = ab.tile([128, TPC], F32, tag="qiT", bufs=2,
                                      name=f"qiT_{nc.next_id()}")
                        pq = mm_ps([128, TPC])
                        for dt in range(4):
                            nc.tensor.matmul(
                                pq[:],
                                (idxq_sb[:, dt, hp * 128:(hp + 1) * 128]),
                                (hT_own[:, dt, :]), start=dt == 0,
                                stop=dt == 3)
                        nc.scalar.copy(qiT[:], pq[:])
                        qiT_l.append(qiT)
                    qhT_l = []
                    for hp in range(H // 2):
                        qhT = ab.tile([128, TPC], F32, tag="qhT", bufs=4,
                                      name=f"qhT_{nc.next_id()}")
                        pq = mm_ps([128, TPC])
                        for dt in range(4):
                            nc.tensor.matmul(
                                pq[:],
                                (wqkv_sb[:, dt, hp * 128:(hp + 1) * 128]),
                                (hT_own[:, dt, :]), start=dt == 0,
                                stop=dt == 3)
                        nc.scalar.copy(qhT[:], pq[:])
                        qhT_l.append(qhT)
                    ag1_in = dr.tile([D, TPC], F32, tag="ag1i",
                                     name=f"ag1i_{l}")
                    ag1in_dma = nc.sync.dma_start(
                        ag1_in.rearrange("(d p) t -> p d t", p=128), hT_own[:])
                    # prioritize the AllGather input over bulky weight loads
                    deps = [idxq_dma, wqkv_dma, wo_dma,
                            ikh_dma, ikl_dma, kvh_dma, kvl_dma]
                    if l == 0:
                        deps.append(c01_dma)
                    for _d in deps:
                        _add_dep_helper(_d.ins, ag1in_dma.ins, sync=True,
                                        reason="defer weight DMA behind ag1")
                    ag1_out = dr.tile([4 * D, TPC], F32, tag="ag1o",
                                      name=f"ag1o_{l}")
                    nc.gpsimd.collective_compute(
                        "AllGather", ALU.bypass,
                        ins=[ag1_in[:]], outs=[ag1_out[:]],
                        replica_groups=[[0, 1, 2, 3], [4, 5, 6, 7]])
                    hT_b = ab.tile([128, 4, S], F32, tag="hT_b",
                                   name=f"hTb_{l}")
                    for r in range(4):
                        nc.sync.dma_start(
                            hT_b[:, :, r * TPC:(r + 1) * TPC],
                            ag1_out[r * D:(r + 1) * D].rearrange(
                                "(d p) t -> p d t", p=128))
                    bh = ab.tile([128, 4, S], BF16, tag="bh",
                                 name=f"bh_{l}")
                    nc.vector.tensor_copy(out=bh[:], in_=hT_b[:])
                    bl = ab.tile([128, 4, S], BF16, tag="bl",
                                 name=f"bl_{l}")
                    nc.gpsimd.tensor_tensor(out=bl[:], in0=hT_b[:],
                                            in1=bh[:], op=ALU.subtract)
                    dump(f"hTb{l}", hT_b[:])

                    # ---- lightning indexer scores -> vals ----
                    for qt in range(2):
                        for ch in range(2):
                            nc.vector.tensor_scalar(
                                out=vals[:, qt, ch * 512:(ch + 1) * 512],
                                in0=c01[:, qt, ch * 512:(ch + 1) * 512],
                                scalar1=1e9, scalar2=-1e9,
                                op0=ALU.mult, op1=ALU.add)
                    for hp in range(HI // 2):
                        qiT = qiT_l[hp]
                        kiT = ab.tile([128, S], F32, tag="kiT", bufs=2,
                                      name=f"kiT_{nc.next_id()}")
                        for ch in range(2):
                            pk = mm_ps([128, 512])
                            nmm = 0
                            for ww, bb in ((ikh, bh), (ikh, bl), (ikl, bh)):
                                for dt in range(4):
                                    nc.tensor.matmul(
                                        pk[:],
                                        ww[:, dt, hp * 128:(hp + 1) * 128],
                                        bb[:, dt, ch * 512:(ch + 1) * 512],
                                        start=nmm == 0, stop=nmm == 11)
                                    nmm += 1
                            nc.scalar.copy(
                                kiT[:, ch * 512:(ch + 1) * 512], pk[:])
                        for hh in range(2):
                            h = hp * 2 + hh
                            for qt in range(2):
                                for ch in range(2):
                                    pv = mm_ps([128, 512])
                                    nc.tensor.matmul(
                                        pv[:],
                                        (qiT[hh * 64:(hh + 1) * 64,
                                               qt * 128:(qt + 1) * 128]),
                                        (kiT[hh * 64:(hh + 1) * 64,
                                               ch * 512:(ch + 1) * 512]),
                                        start=True, stop=True)
                                    rl = ab.tile([128, 512], F32, tag="rl",
                                                 bufs=3,
                                                 name=f"rl_{nc.next_id()}")
                                    nc.scalar.activation(rl[:], pv[:], AF.Relu)
                                    dst = vals[:, qt, ch * 512:(ch + 1) * 512]
                                    nc.vector.scalar_tensor_tensor(
                                        out=dst, in0=rl[:],
                                        scalar=float(signs[l][h]), in1=dst,
                                        op0=ALU.mult, op1=ALU.add)

                    # ---- top-K threshold: per-row binary search ----
                    lo = sml.tile([128, 2], F32, tag="lo", name=f"lo_{l}")
                    hi = sml.tile([128, 2], F32, tag="hi", name=f"hi_{l}")
                    for qt in range(2):
                        nc.vector.tensor_reduce(
                            out=hi[:, qt:qt + 1], in_=vals[:, qt, :],
                            axis=AX.X, op=ALU.max)
                        msk = ab.tile([128, S], F32, tag="msk", bufs=1,
                                      name=f"msk_{nc.next_id()}")
                        nc.vector.tensor_tensor(
                            out=msk[:], in0=vals[:, qt, :], in1=c01[:, qt, :],
                            op=ALU.mult)
                        nc.vector.tensor_reduce(
                            out=lo[:, qt:qt + 1], in_=msk[:], axis=AX.X,
                            op=ALU.min)
                    counts = sml.tile([128, 2], F32, tag="cnt",
                                      name=f"cnt_{l}")
                    for it in range(N_ITERS):
                        mid = sml.tile([128, 2], F32, tag="mid",
                                       name=f"mid_{nc.next_id()}")
                        nc.vector.tensor_tensor(
                            out=mid[:], in0=lo[:], in1=hi[:], op=ALU.add)
                        nc.vector.tensor_scalar(
                            out=mid[:], in0=mid[:], scalar1=0.5, scalar2=None,
                            op0=ALU.mult)
                        for qt in range(2):
                            junk = ab.tile([128, S], BF16, tag="junk", bufs=1,
                                           name=f"jk_{nc.next_id()}")
                            nc.vector.tensor_scalar(
                                out=junk[:], in0=vals[:, qt, :],
                                scalar1=mid[:, qt:qt + 1], scalar2=0.0,
                                op0=ALU.is_ge, op1=ALU.add,
                                accum_out=counts[:, qt:qt + 1])
                        hit = sml.tile([128, 2], I32, tag="hit",
                                       name=f"hit_{nc.next_id()}")
                        nc.vector.tensor_scalar(
                            out=hit[:], in0=counts[:], scalar1=float(K),
                            scalar2=None, op0=ALU.is_ge)
                        nc.vector.copy_predicated(lo[:], hit[:], mid[:])
                        nhit = sml.tile([128, 2], I32, tag="nhit",
                                        name=f"nh_{nc.next_id()}")
                        nc.vector.tensor_scalar(
                            out=nhit[:], in0=counts[:], scalar1=float(K),
                            scalar2=None, op0=ALU.is_lt)
                        nc.vector.copy_predicated(hi[:], nhit[:], mid[:])

                    for qt in range(2):
                        nc.vector.tensor_scalar(
                            out=ind[:, qt, :], in0=vals[:, qt, :],
                            scalar1=lo[:, qt:qt + 1], scalar2=None,
                            op0=ALU.is_ge)
                    dump(f"vals{l}", vals[:])
                    dump(f"lo{l}", lo[:])
                    dump(f"ind{l}", ind[:])
                    for qt in range(2):
                        for kt in range(8):
                            transpose_128(
                                indT[:, kt, qt * 128:(qt + 1) * 128],
                                ind[:, qt, kt * 128:(kt + 1) * 128],
                                dtype=BF16)

                    # ---- attention (f32r matmuls) ----
                    v_sb = ab.tile([128, 8, H, DH + 1], F32, tag="v_sb",
                                   name=f"v_{l}")
                    nc.sync.dma_start(
                        v_sb[:, :, :, DH:DH + 1],
                        one_p.rearrange("p (a b o) -> p a b o", a=8, o=1))
                    for kt in range(8):
                        pvv = mm_ps([128, 512])
                        nmm = 0
                        for bb, ww in ((bh, kvh), (bl, kvh), (bh, kvl)):
                            for dt in range(4):
                                nc.tensor.matmul(
                                    pvv[:],
                                    bb[:, dt, kt * 128:(kt + 1) * 128],
                                    ww[:, 1, dt, :], start=nmm == 0,
                                    stop=nmm == 11)
                                nmm += 1
                        nc.vector.tensor_copy(
                            out=v_sb[:, kt, :, 0:DH],
                            in_=pvv[:].rearrange("p (h d) -> p h d", h=H))

                    # swapped PV: paT[dh, q] = sum_k v[k, dh] * p[k, q]
                    # gives attention output pre-transposed for the wo proj;
                    # row DH is the softmax denominator.
                    aoT = ab.tile([DH, 2, H, 128], F32, tag="aoT",
                                  name=f"aoT_{l}")
                    for hp in range(H // 2):
                        qhT = qhT_l[hp]
                        khT = ab.tile([128, S], F32, tag="khT", bufs=2,
                                      name=f"khT_{nc.next_id()}")
                        for ch in range(2):
                            pk = mm_ps([128, 512])
                            nmm = 0
                            for ww, bb in ((kvh, bh), (kvh, bl), (kvl, bh)):
                                for dt in range(4):
                                    nc.tensor.matmul(
                                        pk[:],
                                        ww[:, 0, dt,
                                           hp * 128:(hp + 1) * 128],
                                        bb[:, dt, ch * 512:(ch + 1) * 512],
                                        start=nmm == 0, stop=nmm == 11)
                                    nmm += 1
                            nc.scalar.copy(
                                khT[:, ch * 512:(ch + 1) * 512], pk[:])
                        for hh in range(2):
                            h = hp * 2 + hh
                            pa = mm_ps([DH + 1, 2, 128], pool=aops, tag="ao")
                            for kt in range(8):
                                pl = mm_ps([128, TPC])
                                nc.tensor.matmul(
                                    pl[:],
                                    (khT[hh * 64:(hh + 1) * 64,
                                           kt * 128:(kt + 1) * 128]),
                                    (qhT[hh * 64:(hh + 1) * 64, :]),
                                    start=True, stop=True)
                                pT = ab.tile([128, TPC], F32, tag="pT", bufs=3,
                                             name=f"pT_{nc.next_id()}")
                                nc.scalar.activation(pT[:], pl[:], AF.Exp)
                                nc.gpsimd.tensor_tensor(
                                    out=pT[:], in0=pT[:], in1=indT[:, kt, :],
                                    op=ALU.mult)
                                nc.tensor.matmul(
                                    pa[:].rearrange("m a b -> m (a b)"),
                                    (v_sb[:, kt, h, :]),
                                    (pT[:]), start=kt == 0, stop=kt == 7)
                            # normalize: divide the 64 value rows by the
                            # denominator row (DH), broadcast via K=1 matmul
                            rrow = ab.tile([1, TPC], F32, tag="rrow", bufs=2,
                                           name=f"rrow_{nc.next_id()}")
                            with nc.allow_low_precision(
                                    reason="f32r rounding of softmax denom"):
                                nc.vector.reciprocal(
                                    rrow[:].rearrange("m (a b) -> m a b",
                                                      a=2),
                                    pa[DH:DH + 1, :, :])
                            rb = pst.tile([DH, TPC], F32, tag="tr",
                                          name=f"rb_{nc.next_id()}")
                            nc.tensor.matmul(
                                rb[:], (ones64[:]), (rrow[:]),
                                start=True, stop=True)
                            rbs = ab.tile([DH, TPC], F32, tag="rbs", bufs=3,
                                          name=f"rbs_{nc.next_id()}")
                            nc.scalar.copy(rbs[:], rb[:])
                            nc.vector.tensor_tensor(
                                out=aoT[:, :, h, :],
                                in0=pa[0:DH, :, :],
                                in1=rbs[:].rearrange("m (a b) -> m a b", a=2),
                                op=ALU.mult)
                    dump(f"aoT{l}", aoT[:])

                    for qt in range(2):
                        po = mm_ps([128, D])
                        for h in range(H):
                            nc.tensor.matmul(
                                po[:],
                                (aoT[:, qt, h, :]),
                                (wo_sb[:, h, :]),
                                start=h == 0, stop=h == 7)
                        nc.vector.tensor_tensor(
                            out=x_own[:, qt, :], in0=x_own[:, qt, :],
                            in1=po[:], op=ALU.add)
                dump(f"xattn{l}", x_own[:])

                # ---- MoE ----
                mh = wrk.tile([128, 2, D], F32, tag="xh", bufs=1,
                              name=f"mh_{l}")
                for qt in range(2):
                    normalize(x_own[:, qt, :], mh[:, qt, :])

                with (
                    tc.tile_pool(name=f"moe{l}", bufs=1) as mb,
                    tc.tile_pool(name=f"moew{l}", bufs=2) as mw,
                    tc.tile_pool(name=f"moeps{l}", bufs=2,
                                 space="PSUM") as mps,
                ):
                    rw_sb = mb.tile([128, 4, E], F32, tag="rw",
                                    name=f"rw_{l}")
                    nc.sync.dma_start(
                        rw_sb[:], rw_p[l].rearrange("(d p) n -> p d n", p=128))
                    mT_own = mb.tile([128, 4, TPC], F32, tag="mT_own",
                                     name=f"mTo_{l}")
                    for qt in range(2):
                        for dt in range(4):
                            transpose_128(
                                mT_own[:, dt, qt * 128:(qt + 1) * 128],
                                mh[:, qt, dt * 128:(dt + 1) * 128])

                    gate = wrk.tile([128, 2, E], F32, tag="gate", bufs=1,
                                    name=f"gate_{l}")
                    for qt in range(2):
                        pr = mm_ps([128, E])
                        for dt in range(4):
                            nc.tensor.matmul(
                                pr[:],
                                (mT_own[:, dt, qt * 128:(qt + 1) * 128]),
                                (rw_sb[:, dt, :]), start=dt == 0,
                                stop=dt == 3)
                        rl_ = sml.tile([128, E], F32, tag="rlog",
                                       name=f"rlog_{nc.next_id()}")
                        nc.vector.tensor_copy(out=rl_[:], in_=pr[:])
                        m1 = sml.tile([128, 1], F32, tag="m1",
                                      name=f"m1_{nc.next_id()}")
                        nc.vector.tensor_reduce(out=m1[:], in_=rl_[:],
                                                axis=AX.X, op=ALU.max)
                        t1 = sml.tile([128, E], F32, tag="t1",
                                      name=f"t1_{nc.next_id()}")
                        nc.vector.tensor_scalar(
                            out=t1[:], in0=rl_[:], scalar1=m1[:],
                            scalar2=None, op0=ALU.is_equal)
                        lp = sml.tile([128, E], F32, tag="lp",
                                      name=f"lp_{nc.next_id()}")
                        nc.vector.scalar_tensor_tensor(
                            out=lp[:], in0=t1[:], scalar=-1e30, in1=rl_[:],
                            op0=ALU.mult, op1=ALU.add)
                        m2 = sml.tile([128, 1], F32, tag="m2",
                                      name=f"m2_{nc.next_id()}")
                        nc.vector.tensor_reduce(out=m2[:], in_=lp[:],
                                                axis=AX.X, op=ALU.max)
                        dd = sml.tile([128, 1], F32, tag="dd",
                                      name=f"dd_{nc.next_id()}")
                        nc.vector.tensor_tensor(out=dd[:], in0=m1[:],
                                                in1=m2[:], op=ALU.subtract)
                        g1 = sml.tile([128, 1], F32, tag="g1",
                                      name=f"g1_{nc.next_id()}")
                        nc.scalar.activation(g1[:], dd[:], AF.Sigmoid)
                        g2 = sml.tile([128, 1], F32, tag="g2",
                                      name=f"g2_{nc.next_id()}")
                        nc.vector.tensor_scalar(
                            out=g2[:], in0=g1[:], scalar1=-1.0, scalar2=1.0,
                            op0=ALU.mult, op1=ALU.add)
                        t2 = sml.tile([128, E], F32, tag="t2",
                                      name=f"t2_{nc.next_id()}")
                        nc.vector.tensor_scalar(
                            out=t2[:], in0=lp[:], scalar1=m2[:], scalar2=None,
                            op0=ALU.is_equal)
                        nc.vector.tensor_scalar(
                            out=gate[:, qt, :], in0=t1[:], scalar1=g1[:],
                            scalar2=None, op0=ALU.mult)
                        nc.vector.scalar_tensor_tensor(
                            out=gate[:, qt, :], in0=t2[:], scalar=g2[:],
                            in1=gate[:, qt, :], op0=ALU.mult, op1=ALU.add)
                    dump(f"gate{l}", gate[:])

                    # local experts: every core runs its own 256 tokens
                    # through all 8 experts (f32r weights streamed from HBM),
                    # gate-weighted accumulation -- no collectives
                    yacc = mb.tile([128, 2, D], F32, tag="yacc",
                                   name=f"yacc_{l}")
                    m1 = mb.tile([128, 4, TPC], FP16, tag="m1",
                                 name=f"m1_{l}")
                    nc.vector.tensor_copy(out=m1[:], in_=mT_own[:])
                    if l == 0:
                        m2 = mb.tile([128, 4, TPC], FP16, tag="m2",
                                     name=f"m2_{l}")
                        nc.gpsimd.tensor_tensor(
                            out=m2[:], in0=mT_own[:], in1=m1[:],
                            op=ALU.subtract)
                    for e in range(E):
                        w1h = mw.tile([128, 4, F], FP16, tag="w1h",
                                      name=f"w1h_{l}_{e}")
                        nc.sync.dma_start(
                            w1h[:],
                            w1_p[l, e].rearrange("(d p) f -> p d f", p=128))
                        w2h = mw.tile([128, 16, D], FP16, tag="w2h",
                                      name=f"w2h_{l}_{e}")
                        w2h_dma = nc.sync.dma_start(
                            w2h[:],
                            w2_p[l, e].rearrange("(f p) d -> p f d", p=128))
                        if l == 1 and e == E - 1:
                            last_wdma[0] = w2h_dma
                        if l == 0:
                            w1lo = mw.tile([128, 4, F], FP16, tag="w1l",
                                           name=f"w1l_{e}")
                            nc.sync.dma_start(
                                w1lo[:],
                                w1l_p[e].rearrange("(d p) f -> p d f", p=128))
                            w2lo = mw.tile([128, 16, D], FP16, tag="w2l",
                                           name=f"w2l_{e}")
                            nc.sync.dma_start(
                                w2lo[:],
                                w2l_p[e].rearrange("(f p) d -> p f d", p=128))
                        h1h = mb.tile([128, 16, TPC], FP16, tag="h1h",
                                      bufs=2,
                                      name=f"h1h_{nc.next_id()}")
                        if l == 0:
                            h1l = mb.tile([128, 16, TPC], FP16, tag="h1l",
                                          bufs=1, name=f"h1l_{nc.next_id()}")
                        for ft in range(16):
                            ph = mm_ps([128, TPC],
                                       pool=mps if ft % 3 == 2 else None)
                            if l == 0:
                                passes = ([(w1h, m1)] * 4 + [(w1h, m2)] * 4
                                          + [(w1lo, m1)] * 4)
                                for i, (ww, mm) in enumerate(passes):
                                    dt = i % 4
                                    nc.tensor.matmul(
                                        ph[:],
                                        ww[:, dt, ft * 128:(ft + 1) * 128],
                                        mm[:, dt, :],
                                        start=i == 0, stop=i == 11)
                            else:
                                for dt in range(4):
                                    nc.tensor.matmul(
                                        ph[:],
                                        w1h[:, dt, ft * 128:(ft + 1) * 128],
                                        m1[:, dt, :],
                                        start=dt == 0, stop=dt == 3)
                            if l == 0:
                                h1f = wrk.tile([128, TPC], F32, tag="h1f",
                                               name=f"h1f_{nc.next_id()}")
                                nc.scalar.activation(h1f[:], ph[:],
                                                     AF.Gelu_apprx_tanh)
                                nc.vector.tensor_copy(out=h1h[:, ft, :],
                                                      in_=h1f[:])
                                nc.gpsimd.tensor_tensor(
                                    out=h1l[:, ft, :], in0=h1f[:],
                                    in1=h1h[:, ft, :], op=ALU.subtract)
                            else:
                                nc.scalar.activation(h1h[:, ft, :], ph[:],
                                                     AF.Gelu_apprx_tanh)
                        for qt in range(2):
                            ph2 = mm_ps([128, D])
                            if l == 0:
                                nmm = 0
                                for ft in range(16):
                                    for hh, ww in ((h1h, w2h), (h1l, w2h),
                                                   (h1h, w2lo)):
                                        nc.tensor.matmul(
                                            ph2[:],
                                            hh[:, ft,
                                               qt * 128:(qt + 1) * 128],
                                            ww[:, ft, :], start=nmm == 0,
                                            stop=nmm == 47)
                                        nmm += 1
                            else:
                                for ft in range(16):
                                    nc.tensor.matmul(
                                        ph2[:],
                                        h1h[:, ft, qt * 128:(qt + 1) * 128],
                                        w2h[:, ft, :], start=ft == 0,
                                        stop=ft == 15)
                            if e == 0:
                                nc.vector.tensor_scalar(
                                    out=yacc[:, qt, :], in0=ph2[:],
                                    scalar1=gate[:, qt, e:e + 1],
                                    scalar2=None, op0=ALU.mult)
                            else:
                                nc.vector.scalar_tensor_tensor(
                                    out=yacc[:, qt, :], in0=ph2[:],
                                    scalar=gate[:, qt, e:e + 1],
                                    in1=yacc[:, qt, :],
                                    op0=ALU.mult, op1=ALU.add)
                    dump(f"yacc{l}", yacc[:])
                    for qt in range(2):
                        nc.vector.tensor_tensor(
                            out=x_own[:, qt, :], in0=x_own[:, qt, :],
                            in1=yacc[:, qt, :], op=ALU.add)
                dump(f"xmoe{l}", x_own[:])

            # =======================================================
            # final LN + vocab projection (column-sharded, bf16)
            # =======================================================
            with tc.tile_pool(name="voc", bufs=1) as vb:
                xf = wrk.tile([128, 2, D], F32, tag="xh", bufs=1, name="xf")
                for qt in range(2):
                    normalize(x_own[:, qt, :], xf[:, qt, :])
                xfb = vb.tile([128, 2, D], BF16, tag="xfb", name="xfb")
                nc.scalar.copy(xfb[:], xf[:])
                xfT_own = vb.tile([128, 4, TPC], BF16, tag="xfT", name="xfT")
                for qt in range(2):
                    for dt in range(4):
                        transpose_128(xfT_own[:, dt, qt * 128:(qt + 1) * 128],
                                      xfb[:, qt, dt * 128:(dt + 1) * 128],
                                      dtype=BF16)
                # AllGather in two token halves: vocab matmuls for the
                # first half overlap the second transfer
                xfT_full = vb.tile([128, 4, NC, TPC], BF16, tag="xfT_full",
                                   name="xfTf")
                for half in range(2):
                    agi = dr.tile([D, 128], BF16, tag=f"ag3i{half}",
                                  name=f"ag3i{half}")
                    nc.sync.dma_start(
                        agi.rearrange("(d p) t -> p d t", p=128),
                        xfT_own[:, :, half * 128:(half + 1) * 128])
                    ago = dr.tile([NC * D, 128], BF16, addr_space="Shared",
                                  tag=f"ag3o{half}", name=f"ag3o{half}")
                    nc.gpsimd.collective_compute(
                        "AllGather", ALU.bypass, ins=[agi[:]],
                        outs=[ago[:]], replica_groups=[list(range(NC))])
                    for r in range(NC):
                        nc.sync.dma_start(
                            xfT_full[:, :, r,
                                     half * 128:(half + 1) * 128],
                            ago[r * D:(r + 1) * D].rearrange(
                                "(d p) t -> p d t", p=128))

                NVC = 8
                CW = VPC // NVC  # 500
                for vc in range(NVC):
                    owc = vb.tile([128, 4, CW], BF16, tag="outw", bufs=2,
                                  name=f"owc_{vc}")
                    owc_dma = nc.sync.dma_start(
                        owc[:],
                        outw_p[:, vc * CW:(vc + 1) * CW].rearrange(
                            "(d p) v -> p d v", p=128))
                    if vc < 2 and last_wdma[0] is not None:
                        _add_dep_helper(owc_dma.ins, last_wdma[0].ins,
                                        sync=True,
                                        reason="vocab weights after experts")
                    for qt in (list(range(0, 16, 2))
                               + list(range(1, 16, 2))):
                        r, hf = qt // 2, qt % 2
                        pv = mm_ps([128, CW])
                        for dt in range(4):
                            nc.tensor.matmul(
                                pv[:],
                                xfT_full[:, dt, r, hf * 128:(hf + 1) * 128],
                                owc[:, dt, :], start=dt == 0, stop=dt == 3)
                        oc = vb.tile([128, CW], BF16, tag="oc", bufs=6,
                                     name=f"oc_{nc.next_id()}")
                        if qt % 2 == 0:
                            nc.vector.tensor_copy(out=oc[:], in_=pv[:])
                        else:
                            nc.scalar.copy(oc[:], pv[:])
                        nc.sync.dma_start(
                            out_p[qt * 128:(qt + 1) * 128,
                                  vc * CW:(vc + 1) * CW], oc[:])

    nc.compile()
    return nc


# -------------------------------------------------------------- host side --
_CACHE = {}
_LAST_IN_MAPS = None


def _np(x, dt=np.float32):
    return np.ascontiguousarray(np.asarray(x), dtype=dt)


def kernel(**inputs):
    ids = _np(inputs["input_ids"], np.int64).reshape(B, S)
    tok_emb = _np(inputs["tok_emb"])
    pos_emb = _np(inputs["pos_emb"])
    ln1_g, ln1_b = _np(inputs["ln1_g"]), _np(inputs["ln1_b"])
    ln2_g, ln2_b = _np(inputs["ln2_g"]), _np(inputs["ln2_b"])
    lnf_g, lnf_b = _np(inputs["lnf_g"]), _np(inputs["lnf_b"])
    idx_qw, idx_qb = _np(inputs["idx_qw"]), _np(inputs["idx_qb"])
    idx_kw, idx_kb = _np(inputs["idx_kw"]), _np(inputs["idx_kb"])
    idx_hw = _np(inputs["idx_hw"])
    wq, bq = _np(inputs["wq"]), _np(inputs["bq"])
    wk, bk = _np(inputs["wk"]), _np(inputs["bk"])
    wv, bv = _np(inputs["wv"]), _np(inputs["bv"])
    wo, bo = _np(inputs["wo"]), _np(inputs["bo"])
    router_w, router_b = _np(inputs["router_w"]), _np(inputs["router_b"])
    e_w1, e_b1 = _np(inputs["e_w1"]), _np(inputs["e_b1"])
    e_w2, e_b2 = _np(inputs["e_w2"]), _np(inputs["e_b2"])
    out_w, out_b = _np(inputs["out_w"]), _np(inputs["out_b"])

    for nm, b in [("ln1_b", ln1_b), ("ln2_b", ln2_b), ("lnf_b", lnf_b),
                  ("idx_qb", idx_qb), ("idx_kb", idx_kb), ("bq", bq),
                  ("bk", bk), ("bv", bv), ("bo", bo), ("router_b", router_b),
                  ("e_b1", e_b1), ("e_b2", e_b2), ("out_b", out_b)]:
        assert np.abs(b).max() == 0.0, f"nonzero bias {nm} unsupported"

    x0 = tok_emb[ids.reshape(-1)] + np.tile(pos_emb[:S], (B, 1))  # [T, D]

    scale = 1.0 / np.sqrt(DH)
    idxq_f = idx_qw * ln1_g[:, :, None]
    signs = np.sign(idx_hw)
    signs[signs == 0] = 1.0
    for l in range(L):
        for h in range(HI):
            idxq_f[l][:, h * DI:(h + 1) * DI] *= abs(idx_hw[l, h])
    idxk_f = idx_kw * ln1_g[:, :, None]
    wq_f = wq * ln1_g[:, :, None] * scale
    wk_f = wk * ln1_g[:, :, None]
    wv_f = wv * ln1_g[:, :, None]
    wqkv = np.stack([wq_f, wk_f, wv_f, wo], axis=1)  # [L, 4, D, D]
    rw_f = router_w * ln2_g[:, :, None]
    w1_f = e_w1 * ln2_g[:, None, :, None]            # [L, E, D, F]
    outw_f = out_w * lnf_g[:, None]

    if "nc" not in _CACHE:
        _CACHE["nc"] = _build(signs)
    nc = _CACHE["nc"]

    ident_b = np.eye(128, dtype=bf16)
    ident_f = np.eye(128, dtype=np.float32)
    idxk64 = idxk_f.astype(np.float64)
    idxk_h = idxk64.astype(bf16)
    idxk_l = (idxk64 - idxk_h.astype(np.float64)).astype(bf16)
    wkv64 = np.stack([wk_f, wv_f], axis=1).astype(np.float64)
    wkv_h = wkv64.astype(bf16)
    wkv_l = (wkv64 - wkv_h.astype(np.float64)).astype(bf16)
    w1_hi = w1_f.astype(np.float16)
    w2_hi = e_w2.astype(np.float16)
    w1_lo = (w1_f[0] - w1_hi[0].astype(np.float64)).astype(np.float16)
    w2_lo = (e_w2[0] - w2_hi[0].astype(np.float64)).astype(np.float16)
    in_maps = []
    for c in range(NC):
        rows = slice(c * TPC, (c + 1) * TPC)
        p = np.arange(S)[(c % 4) * TPC:(c % 4 + 1) * TPC]
        c01 = (np.arange(S)[None, :] <= p[:, None]).astype(np.float32)
        in_maps.append({
            "x0": x0[rows].astype(np.float32),
            "c01": c01,
            "idxq": idxq_f.astype(np.float32),
            "idxk": idxk_f.astype(np.float32),
            "wqkv": wqkv.astype(np.float32),
            "rw": rw_f.astype(np.float32),
            "w1": w1_hi,
            "w2": w2_hi,
            "w1l": w1_lo,
            "w2l": w2_lo,
            "idxkh": idxk_h,
            "idxkl": idxk_l,
            "wkvh": wkv_h,
            "wkvl": wkv_l,
            "outw": outw_f[:, c * VPC:(c + 1) * VPC].astype(bf16),
            "idb": ident_b,
            "idf": ident_f,
            "one": np.ones((128, 64), np.float32),
        })

    global _LAST_IN_MAPS
    _LAST_IN_MAPS = in_maps
    res = run_bass_kernel_spmd(nc, in_maps, core_ids=list(range(NC)))
    outs = [np.asarray(res.results[c]["out"]).astype(np.float32)
            for c in range(NC)]
    full = np.concatenate(outs, axis=1).reshape(B, S, V)
    return np.ascontiguousarray(full, dtype=np.float32)


if __name__ == "__main__":
    import reference
    inp = {k: np.asarray(v) for k, v in reference.setup_inputs().items()}
    got = kernel(**inp)
    print("kernel output", got.shape, got.dtype)
